# revision 22
# baseline (speedup 1.0000x reference)
"""Two-layer GCN (PyG GCNConv style) on 8 Trainium2 NeuronCores.

Math (reference):
    src,dst += self-loops; deg = indeg(dst)+1 ; dinv = deg^-1/2
    norm_e  = dinv[src]*dinv[dst]
    h  = relu( scatter(norm * (x@W1)[src] -> dst) + b1 )
    out =      scatter(norm * (h@W2)[src] -> dst) + b2

Factorization used here: with P' = dinv (.) (x@W1) = (dinv (.) x)@W1,
    h[d]  = relu( dinv[d] * ( sum_{e->d} P'[src_e] + P'[d] ) + b1 )
    h'    = dinv (.) h
    out[d]= dinv[d] * ( ( sum_{e->d} h'[src_e] + h'[d] ) @ W2 ) + b2
so the per-edge coefficient disappears: the scatter becomes a pure
segment-sum of gathered rows, done as one-hot matmuls on the PE; dinv
scalings ride the per-tile epilogues and the self-loop term is a local
row add.

Sharding: nodes 12500/core (8 cores); edges partitioned by dst core and
sorted by (dst-tile, src-bank); gathers use the int16 dma_gather custom
DMA (4 source banks of 25000 rows); AllGather replicates the per-core
feature chunk between layers. SPMD: one program, per-core data; per
(tile,bank) capacities are padded to the max over cores (x128).
"""

import numpy as np
import ml_dtypes

N = 100000
NCORES = 8
M = N // NCORES          # 12500 nodes per core
IND, HID, OUTD = 256, 128, 237
P = 128
NT = (M + P - 1) // P    # 98 dst tiles per core; last tile 84 rows
LAST_ROWS = M - (NT - 1) * P
NB = 4
BANK = N // NB           # 25000 rows per gather bank (int16 idx < 32768)
GT = 4                   # dst tiles per group (gather granularity)
AG_SHARED = True         # AllGather outputs in Shared scratchpad
AGG_BUFS = 2             # pipelining depth of the aggregation pools
REPEAT = 1               # run the whole pipeline R times (timing experiments)
BANK_ORDER = (1, 2, 3, 0)  # issue async queues first; blocking q0 last

BF16 = ml_dtypes.bfloat16


def _plan(counts):
    """counts: [NCORES, NT, NB] edge counts. Returns the static schedule."""
    U = ((counts.max(axis=0) + 127) // 128) * 128  # [NT, NB]
    groups = []
    idx_off = 0   # int16 elements into the global idx stream
    ch_off = 0    # global dstloc column offset
    ng = (NT + GT - 1) // GT
    for g in range(ng):
        tiles = list(range(g * GT, min((g + 1) * GT, NT)))
        blen = []      # idxs per bank gather call
        gb_choff = []  # chunk offset of each bank segment inside gbuf
        ch = 0
        for b in range(NB):
            L = int(sum(U[t][b] for t in tiles))
            blen.append(L)
            gb_choff.append(ch)
            ch += L // 128
        # per tile: list of (gbuf_chunk, dstloc_col) in matmul order
        tile_chunks = {}
        dcol = ch_off
        for t in tiles:
            lst = []
            for b in range(NB):
                pre = int(sum(U[t2][b] for t2 in tiles if t2 < t)) // 128
                for j in range(int(U[t][b]) // 128):
                    lst.append((gb_choff[b] + pre + j, dcol))
                    dcol += 1
            tile_chunks[t] = lst
        groups.append(
            dict(
                tiles=tiles,
                blen=blen,
                gb_choff=gb_choff,
                CHg=ch,
                idx_off=idx_off,
                dst_off=ch_off,
                tile_chunks=tile_chunks,
            )
        )
        idx_off += sum(blen)
        ch_off = dcol
    return U, groups, idx_off, ch_off


def _host_prep(x, edge_index, W1, b1, W2, b2):
    x = np.asarray(x, dtype=np.float32)
    ei = np.asarray(edge_index).astype(np.int64)
    src, dst = ei[0], ei[1]
    deg = np.bincount(dst, minlength=N).astype(np.float32) + 1.0
    dinv = (1.0 / np.sqrt(deg)).astype(np.float32)

    core = dst // M
    tile_ = (dst % M) // P
    bank = src // BANK
    key = (core * NT + tile_) * NB + bank
    counts = np.bincount(key, minlength=NCORES * NT * NB).reshape(NCORES, NT, NB)
    order = np.argsort(key, kind="stable")
    starts = np.zeros(NCORES * NT * NB + 1, dtype=np.int64)
    starts[1:] = np.cumsum(counts.reshape(-1))

    U, groups, TOTIDX, TOTCH = _plan(counts)

    src_local = (src - bank * BANK).astype(np.int16)
    dst_local = (dst % M) % P

    xT = (x * dinv[:, None]).T.astype(BF16)  # [256, N], row-scaled

    in_maps = []
    for c in range(NCORES):
        idx_all = np.zeros(TOTIDX, dtype=np.int16)
        dst_cols = np.full((TOTCH, P), -1, dtype=np.int32)
        for g in groups:
            seg = g["idx_off"]
            for b in range(NB):
                toff = 0
                for t in g["tiles"]:
                    k = (c * NT + t) * NB + b
                    n = counts[c, t, b]
                    cap = int(U[t][b])
                    if n:
                        e = order[starts[k] : starts[k] + n]
                        idx_all[seg + toff : seg + toff + n] = src_local[e]
                        vals = np.full(cap, -1, dtype=np.int32)
                        vals[:n] = dst_local[e]
                        # dstloc columns of (t,b): bank-major position in
                        # tile_chunks (chunks of banks < b come first)
                        pos0 = int(sum(U[t][b2] for b2 in range(b))) // 128
                        cols = [
                            g["tile_chunks"][t][pos0 + i][1]
                            for i in range(cap // 128)
                        ]
                        dst_cols[cols, :] = vals.reshape(-1, P)
                    toff += cap
                seg += g["blen"][b]
        # wrap idx stream per (g,b): idx k -> partition k%16, col k//16
        blocks = []
        for g in groups:
            seg = g["idx_off"]
            for b in range(NB):
                L = g["blen"][b]
                if L:
                    blocks.append(idx_all[seg : seg + L].reshape(-1, 16).T)
                seg += L
        idxw = np.tile(np.hstack(blocks), (8, 1))  # [128, TOTIDX//16]

        # host-built scatter masks for layer 0, with the dst-side dinv norm
        # baked in: masks[p, ch, d] = dinv[dst] iff the edge in slot
        # (chunk ch, partition p) targets local dst row d, else 0.
        tilebase = np.zeros(TOTCH, dtype=np.int64)
        for g in groups:
            for t in g["tiles"]:
                for (_, col) in g["tile_chunks"][t]:
                    tilebase[col] = t * P
        valid = dst_cols >= 0                              # [TOTCH, P]
        dglob = np.where(valid, c * M + tilebase[:, None] + dst_cols, 0)
        dval = (dinv[dglob] * valid).astype(np.float32)    # [TOTCH, P]
        oh = dst_cols.T[:, :, None] == np.arange(P, dtype=np.int32)[None, None, :]
        masks = (oh * dval.T[:, :, None]).astype(BF16)     # [P, TOTCH, P]

        # local dst rows for the L1 DVE-built one-hot masks
        dstl16 = np.where(valid, dst_cols, -1).astype(np.float32)

        dv = dinv[c * M : (c + 1) * M]
        dvp = np.concatenate([dv, np.ones(NT * P - M, np.float32)])
        in_maps.append(
            {
                "xT": np.ascontiguousarray(xT[:, c * M : (c + 1) * M]),
                "idx": idxw,
                "masks": masks,
                "dstloc": np.ascontiguousarray(dstl16.T).astype(BF16),
                "dinv": np.ascontiguousarray(dvp.reshape(NT, P).T),
                "iota": np.tile(np.arange(P, dtype=np.float32), (P, 1))
                .astype(BF16)
                .reshape(P, 1, P),
                "W1": np.asarray(W1, np.float32).astype(BF16),
                "W2": np.asarray(W2, np.float32).astype(BF16),
                "b1r": np.tile(np.asarray(b1, np.float32), (P, 1)),
                "b2r": np.tile(np.asarray(b2, np.float32), (P, 1)),
            }
        )
    return in_maps, U, groups, TOTIDX, TOTCH


ABLATE = frozenset()


def _build_nc(groups, TOTIDX, TOTCH, ablate=None):
    ablate = ABLATE if ablate is None else ablate
    import concourse.bacc as bacc
    import concourse.mybir as mybir
    import concourse.tile as tile
    from concourse.masks import make_identity

    F32 = mybir.dt.float32
    BF = mybir.dt.bfloat16
    I16 = mybir.dt.int16
    AOP = mybir.AluOpType

    nc = bacc.Bacc(
        "TRN2", target_bir_lowering=False, num_devices=NCORES, num_swdge_queues=4
    )
    xT_d = nc.dram_tensor("xT", [IND, M], BF, kind="ExternalInput")
    idx_d = nc.dram_tensor("idx", [P, TOTIDX // 16], I16, kind="ExternalInput")
    msk_d = nc.dram_tensor("masks", [P, TOTCH, P], BF, kind="ExternalInput")
    dst_d = nc.dram_tensor("dstloc", [P, TOTCH], BF, kind="ExternalInput")
    dinv_d = nc.dram_tensor("dinv", [P, NT], F32, kind="ExternalInput")
    iota_d = nc.dram_tensor("iota", [P, 1, P], BF, kind="ExternalInput")
    w1_d = nc.dram_tensor("W1", [IND, HID], BF, kind="ExternalInput")
    w2_d = nc.dram_tensor("W2", [HID, OUTD], BF, kind="ExternalInput")
    b1_d = nc.dram_tensor("b1r", [P, HID], F32, kind="ExternalInput")
    b2_d = nc.dram_tensor("b2r", [P, OUTD], F32, kind="ExternalInput")
    out_d = nc.dram_tensor("out", [M, OUTD], F32, kind="ExternalOutput")

    CHMAX = max(g["CHg"] for g in groups)
    NCHMAX = max(len(ck) for g in groups for ck in g["tile_chunks"].values())
    IDXWMAX = max(sum(g["blen"]) for g in groups) // 16

    with tile.TileContext(nc) as tc:
        with (
            tc.tile_pool(name="dram", bufs=1, space="DRAM") as dpool,
            tc.tile_pool(name="const", bufs=1) as cp,
            tc.tile_pool(name="resid", bufs=1) as rp,
        ):
            p_chunk = dpool.tile([M, HID], BF)
            pfull = dpool.tile([N, HID], BF, addr_space="Shared" if AG_SHARED else "Local")
            h_chunk = dpool.tile([M, HID], BF)
            hfull = dpool.tile([N, HID], BF, addr_space="Shared" if AG_SHARED else "Local")

            w1a = cp.tile([P, HID], BF)
            w1b = cp.tile([P, HID], BF)
            w2s = cp.tile([HID, OUTD], BF)
            b1r = cp.tile([P, HID], F32)
            b2r = cp.tile([P, OUTD], F32)
            iota = cp.tile([P, 1, P], BF)
            dinv = cp.tile([P, NT], F32)
            zero1 = cp.tile([P, 1], F32)
            ident = cp.tile([P, P], BF)
            nc.sync.dma_start(w1a[:], w1_d[0:P, :])
            nc.sync.dma_start(w1b[:], w1_d[P:IND, :])
            nc.sync.dma_start(w2s[:], w2_d[:])
            nc.sync.dma_start(b1r[:], b1_d[:])
            nc.sync.dma_start(b2r[:], b2_d[:])
            nc.sync.dma_start(iota[:], iota_d[:])
            nc.sync.dma_start(dinv[:], dinv_d[:])
            nc.vector.memset(zero1[:], 0.0)
            make_identity(nc, ident[:])

            # resident per-core feature copies
            # p2_sb = dinv*P' + b1  (the prescaled self term for layer 0)
            p2_sb = rp.tile([P, NT, HID], BF)
            h_sb = rp.tile([P, NT, HID], BF)
            # last tile only has LAST_ROWS valid rows; zero the tail lanes
            nc.vector.memset(p2_sb[:, NT - 1, :], 0.0)
            nc.vector.memset(h_sb[:, NT - 1, :], 0.0)

            # ---------------- stage A: P' = (dinv.x) @ W1 ----------------
            for _rep in range(REPEAT):
              with (
                  nc.named_scope("stageA"),
                  tc.tile_pool(name="sa", bufs=2) as sa,
                  tc.tile_pool(name="psA", bufs=2, space="PSUM") as psA,
              ):
                  for gi_, g_ in enumerate(groups):
                      c0 = g_["tiles"][0] * P
                      c1 = min(c0 + GT * P, M)
                      cols = c1 - c0
                      xa = sa.tile([P, GT * P], BF, tag="xa")
                      xb = sa.tile([P, GT * P], BF, tag="xb")
                      nc.sync.dma_start(xa[:, :cols], xT_d[0:P, c0:c1])
                      nc.sync.dma_start(xb[:, :cols], xT_d[P:IND, c0:c1])
                      tiles = g_["tiles"]
                      slab = sa.tile([P, GT, HID], BF, tag="slab")
                      for j, t in enumerate(tiles):
                          rows = P if t < NT - 1 else LAST_ROWS
                          ps = psA.tile([P, HID], mybir.dt.float32, tag="psA")
                          nc.tensor.matmul(
                              ps[:rows, :], lhsT=xa[:, j * P : j * P + rows],
                              rhs=w1a[:], start=True, stop=False,
                          )
                          nc.tensor.matmul(
                              ps[:rows, :], lhsT=xb[:, j * P : j * P + rows],
                              rhs=w1b[:], start=False, stop=True,
                          )
                          nc.scalar.activation(
                              slab[:rows, j, :], ps[:rows, :],
                              mybir.ActivationFunctionType.Copy, bias=0.0,
                          )
                          pe = sa.tile([P, HID], mybir.dt.float32, tag="pe")
                          nc.vector.tensor_scalar(
                              out=pe[:rows, :], in0=ps[:rows, :],
                              scalar1=dinv[:rows, t : t + 1], scalar2=None,
                              op0=AOP.mult,
                          )
                          nc.vector.tensor_tensor(
                              out=p2_sb[:rows, t, :], in0=pe[:rows, :],
                              in1=b1r[:rows, :], op=AOP.add,
                          )
                      if cols == GT * P:
                          nc.sync.dma_start(
                              p_chunk[c0:c1, :].rearrange("(a p) f -> p a f", p=P),
                              slab[:],
                          )
                      else:
                          for j, t in enumerate(tiles):
                              rows = P if t < NT - 1 else LAST_ROWS
                              r0 = c0 + j * P
                              nc.sync.dma_start(
                                  p_chunk[r0 : r0 + rows, :], slab[:rows, j, :]
                              )

              if "ag" not in ablate:
                  with nc.named_scope("AG1"):
                      nc.gpsimd.collective_compute(
                          "AllGather", mybir.AluOpType.bypass,
                          replica_groups=[list(range(NCORES))],
                          ins=[p_chunk.opt()], outs=[pfull.opt()],
                      )

              # ------------- aggregation layers -------------
              with (
                  tc.tile_pool(name="agg", bufs=AGG_BUFS) as ag,
                  tc.tile_pool(name="psC", bufs=4, space="PSUM") as psC,
                  tc.tile_pool(name="psF", bufs=2, space="PSUM") as psF,
              ):
                  for layer in (0, 1):
                      _lsid, _ = nc.enter_named_scope(f"L{layer}", False)
                      src_full = pfull if layer == 0 else hfull
                      for gi, g in enumerate(groups):
                          W16 = sum(g["blen"]) // 16
                          io = g["idx_off"] // 16
                          idxs = ag.tile([P, IDXWMAX], I16, tag="idxs")
                          nc.scalar.dma_start(idxs[:, :W16], idx_d[:, io : io + W16])
                          if layer == 0:
                              mt = ag.tile([P, CHMAX, P], BF, tag="mt")
                              meng = (nc.scalar, nc.sync)[gi % 2]
                              meng.dma_start(
                                  mt[:, : g["CHg"], :],
                                  msk_d[:, g["dst_off"] : g["dst_off"] + g["CHg"], :],
                              )
                          else:
                              dstl = ag.tile([P, CHMAX], BF, tag="dstl")
                              nc.scalar.dma_start(
                                  dstl[:, : g["CHg"]],
                                  dst_d[:, g["dst_off"] : g["dst_off"] + g["CHg"]],
                              )
                          gbuf = ag.tile([P, CHMAX, HID], BF, tag="gbuf")
                          boffs = np.cumsum([0] + [g["blen"][b] // 16 for b in range(NB)])
                          for b in BANK_ORDER:
                              L = g["blen"][b]
                              if L == 0 or "gather" in ablate:
                                  if L and "gather" in ablate and b == 0:
                                      nc.vector.memset(gbuf[:, :, 0:2], 0.0)
                                  continue
                              nc.gpsimd.dma_gather(
                                  gbuf[:, g["gb_choff"][b] : g["gb_choff"][b] + L // 128, :],
                                  src_full[b * BANK : (b + 1) * BANK, :],
                                  idxs[:, boffs[b] : boffs[b] + L // 16],
                                  L, L, HID, queue_num=b, single_packet=False,
                              )
                          oslab = (
                              None
                              if layer == 0
                              else ag.tile([P, GT, OUTD], mybir.dt.float32, tag="oslab")
                          )
                          for j, t in enumerate(g["tiles"]):
                              rows = P if t < NT - 1 else LAST_ROWS
                              chunks = g["tile_chunks"][t]
                              mmchunks = chunks[:1] if "mm" in ablate else chunks
                              nch_eff = len(mmchunks)
                              if layer == 0:
                                  # ps = dinv.(scatter) with the norm baked into
                                  # the host masks; h' = dinv*relu(ps + p2own)
                                  ps = psC.tile([P, HID], mybir.dt.float32, tag="psC")
                                  for k, (chk, col) in enumerate(mmchunks):
                                      nc.tensor.matmul(
                                          ps[:], lhsT=mt[:, col - g["dst_off"], :],
                                          rhs=gbuf[:, chk, :],
                                          start=(k == 0), stop=(k == nch_eff - 1),
                                      )
                                  e1 = ag.tile([P, HID], mybir.dt.float32, tag="e1")
                                  nc.vector.tensor_tensor(
                                      out=e1[:], in0=ps[:], in1=p2_sb[:, t, :],
                                      op=AOP.add,
                                  )
                                  nc.vector.tensor_scalar(
                                      out=h_sb[:, t, :], in0=e1[:],
                                      scalar1=0.0, scalar2=dinv[:, t : t + 1],
                                      op0=AOP.max, op1=AOP.mult,
                                  )
                              else:
                                  # transposed accumulate: T'[hid,dst] = sum
                                  # gbuf^T.onehot + h'own^T; out = dinv*(T'^T@W2)
                                  nch = len(chunks)
                                  st3 = ag.tile([P, NCHMAX, P], BF, tag="st3")
                                  d0 = chunks[0][1] - g["dst_off"]
                                  nc.vector.tensor_tensor(
                                      out=st3[:, :nch, :],
                                      in0=iota[:].to_broadcast([P, nch, P]),
                                      in1=dstl[:, d0 : d0 + nch]
                                      .rearrange("p (a b) -> p a b", b=1)
                                      .to_broadcast([P, nch, P]),
                                      op=mybir.AluOpType.is_equal,
                                  )
                                  ps = psC.tile([P, P], mybir.dt.float32, tag="psC")
                                  for k, (chk, col) in enumerate(mmchunks):
                                      nc.tensor.matmul(
                                          ps[:], lhsT=gbuf[:, chk, :],
                                          rhs=st3[:, k, :],
                                          start=(k == 0), stop=False,
                                      )
                                  nc.tensor.matmul(
                                      ps[:], lhsT=h_sb[:, t, :], rhs=ident[:],
                                      start=False, stop=True,
                                  )
                                  tt = ag.tile([P, P], BF, tag="tt")
                                  nc.scalar.activation(
                                      tt[:], ps[:],
                                      mybir.ActivationFunctionType.Copy,
                                      bias=0.0,
                                  )
                                  pf = psF.tile([P, OUTD], mybir.dt.float32, tag="psF")
                                  nc.tensor.matmul(
                                      pf[:rows, :], lhsT=tt[:, :rows], rhs=w2s[:],
                                      start=True, stop=True,
                                  )
                                  o1 = ag.tile([P, OUTD], mybir.dt.float32, tag="o1")
                                  nc.vector.tensor_scalar(
                                      out=o1[:rows, :], in0=pf[:rows, :],
                                      scalar1=dinv[:rows, t : t + 1], scalar2=None,
                                      op0=AOP.mult,
                                  )
                                  nc.vector.tensor_tensor(
                                      out=oslab[:rows, j, :], in0=o1[:rows, :],
                                      in1=b2r[:rows, :], op=AOP.add,
                                  )
                          # write the group's output rows
                          c0 = g["tiles"][0] * P
                          c1 = min(c0 + GT * P, M)
                          tgt = h_chunk if layer == 0 else out_d
                          if c1 - c0 == GT * P:
                              src_slab = (
                                  h_sb[:, g["tiles"][0] : g["tiles"][0] + GT, :]
                                  if layer == 0
                                  else oslab[:]
                              )
                              nc.sync.dma_start(
                                  tgt[c0:c1, :].rearrange("(a p) f -> p a f", p=P),
                                  src_slab,
                              )
                          else:
                              for j, t in enumerate(g["tiles"]):
                                  rows = P if t < NT - 1 else LAST_ROWS
                                  r0 = c0 + j * P
                                  ssl = (
                                      h_sb[:rows, t, :]
                                      if layer == 0
                                      else oslab[:rows, j, :]
                                  )
                                  nc.sync.dma_start(tgt[r0 : r0 + rows, :], ssl)
                      nc.leave_named_scope(f"L{layer}", _lsid, False)
                      if layer == 0 and "ag" not in ablate:
                          with nc.named_scope("AG2"):
                              nc.gpsimd.collective_compute(
                                  "AllGather", mybir.AluOpType.bypass,
                                  replica_groups=[list(range(NCORES))],
                                  ins=[h_chunk.opt()], outs=[hfull.opt()],
                              )
    nc.compile()
    return nc


_CACHE = {}


def _get_compiled(x, edge_index, W1, b1, W2, b2):
    in_maps, U, groups, TOTIDX, TOTCH = _host_prep(x, edge_index, W1, b1, W2, b2)
    key = (TOTIDX, TOTCH, ABLATE, GT, AG_SHARED, AGG_BUFS, REPEAT, tuple(int(v) for v in np.asarray(U).reshape(-1)[:32]))
    if key not in _CACHE:
        _CACHE[key] = _build_nc(groups, TOTIDX, TOTCH)
    return _CACHE[key], in_maps


def kernel(x, edge_index, W1, b1, W2, b2):
    from concourse.bass_utils import run_bass_kernel_spmd

    nc, in_maps = _get_compiled(x, edge_index, W1, b1, W2, b2)
    res = run_bass_kernel_spmd(nc, in_maps, core_ids=list(range(NCORES)))
    return np.concatenate([res.results[c]["out"] for c in range(NCORES)], axis=0)



# revision 32
# speedup vs baseline: 1.2119x; 1.2119x over previous
"""Two-layer GCN (PyG GCNConv style) on 8 Trainium2 NeuronCores.

Math (reference):
    src,dst += self-loops; deg = indeg(dst)+1 ; dinv = deg^-1/2
    norm_e  = dinv[src]*dinv[dst]
    h  = relu( scatter(norm * (x@W1)[src] -> dst) + b1 )
    out =      scatter(norm * (h@W2)[src] -> dst) + b2

Factorization used here: with P' = dinv (.) (x@W1) = (dinv (.) x)@W1,
    h[d]  = relu( dinv[d] * ( sum_{e->d} P'[src_e] + P'[d] ) + b1 )
    h'    = dinv (.) h
    out[d]= dinv[d] * ( ( sum_{e->d} h'[src_e] + h'[d] ) @ W2 ) + b2
so the per-edge coefficient disappears: the scatter becomes a pure
segment-sum of gathered rows, done as one-hot matmuls on the PE; dinv
scalings ride the per-tile epilogues and the self-loop term is a local
row add.

Sharding: nodes 12500/core (8 cores); edges partitioned by dst core and
sorted by (dst-tile, src-bank); gathers use the int16 dma_gather custom
DMA (4 source banks of 25000 rows); AllGather replicates the per-core
feature chunk between layers. SPMD: one program, per-core data; per
(tile,bank) capacities are padded to the max over cores (x128).
"""

import numpy as np
import ml_dtypes

N = 100000
NCORES = 8
M = N // NCORES          # 12500 nodes per core
IND, HID, OUTD = 256, 128, 237
P = 128
NT = (M + P - 1) // P    # 98 dst tiles per core; last tile 84 rows
LAST_ROWS = M - (NT - 1) * P
NB = 4
BANK = N // NB           # 25000 rows per gather bank (int16 idx < 32768)
GT = 2                   # dst tiles per group (gather granularity)
AG_SHARED = True         # AllGather outputs in Shared scratchpad
AGG_BUFS = 3             # pipelining depth of the aggregation pools
REPEAT = 1               # run the whole pipeline R times (timing experiments)
BANK_ORDER = (0, 1, 2, 3)  # blocking q0 first: its transfer overlaps q1-3 drain

BF16 = ml_dtypes.bfloat16


def _plan(counts):
    """counts: [NCORES, NT, NB] edge counts. Returns the static schedule."""
    U = ((counts.max(axis=0) + 127) // 128) * 128  # [NT, NB]
    groups = []
    idx_off = 0   # int16 elements into the global idx stream
    ch_off = 0    # global dstloc column offset
    ng = (NT + GT - 1) // GT
    for g in range(ng):
        tiles = list(range(g * GT, min((g + 1) * GT, NT)))
        blen = []      # idxs per bank gather call
        gb_choff = []  # chunk offset of each bank segment inside gbuf
        ch = 0
        for b in range(NB):
            L = int(sum(U[t][b] for t in tiles))
            blen.append(L)
            gb_choff.append(ch)
            ch += L // 128
        # per tile: list of (gbuf_chunk, dstloc_col) in matmul order
        tile_chunks = {}
        dcol = ch_off
        for t in tiles:
            lst = []
            for b in range(NB):
                pre = int(sum(U[t2][b] for t2 in tiles if t2 < t)) // 128
                for j in range(int(U[t][b]) // 128):
                    lst.append((gb_choff[b] + pre + j, dcol))
                    dcol += 1
            tile_chunks[t] = lst
        groups.append(
            dict(
                tiles=tiles,
                blen=blen,
                gb_choff=gb_choff,
                CHg=ch,
                idx_off=idx_off,
                dst_off=ch_off,
                tile_chunks=tile_chunks,
            )
        )
        idx_off += sum(blen)
        ch_off = dcol
    return U, groups, idx_off, ch_off


def _host_prep(x, edge_index, W1, b1, W2, b2):
    x = np.asarray(x, dtype=np.float32)
    ei = np.asarray(edge_index).astype(np.int64)
    src, dst = ei[0], ei[1]
    deg = np.bincount(dst, minlength=N).astype(np.float32) + 1.0
    dinv = (1.0 / np.sqrt(deg)).astype(np.float32)

    core = dst // M
    tile_ = (dst % M) // P
    bank = src // BANK
    key = (core * NT + tile_) * NB + bank
    counts = np.bincount(key, minlength=NCORES * NT * NB).reshape(NCORES, NT, NB)
    order = np.argsort(key, kind="stable")
    starts = np.zeros(NCORES * NT * NB + 1, dtype=np.int64)
    starts[1:] = np.cumsum(counts.reshape(-1))

    U, groups, TOTIDX, TOTCH = _plan(counts)

    src_local = (src - bank * BANK).astype(np.int16)
    dst_local = (dst % M) % P

    xT = (x * dinv[:, None]).T.astype(BF16)  # [256, N], row-scaled

    in_maps = []
    for c in range(NCORES):
        idx_all = np.zeros(TOTIDX, dtype=np.int16)
        dst_cols = np.full((TOTCH, P), -1, dtype=np.int32)
        for g in groups:
            seg = g["idx_off"]
            for b in range(NB):
                toff = 0
                for t in g["tiles"]:
                    k = (c * NT + t) * NB + b
                    n = counts[c, t, b]
                    cap = int(U[t][b])
                    if n:
                        e = order[starts[k] : starts[k] + n]
                        idx_all[seg + toff : seg + toff + n] = src_local[e]
                        vals = np.full(cap, -1, dtype=np.int32)
                        vals[:n] = dst_local[e]
                        # dstloc columns of (t,b): bank-major position in
                        # tile_chunks (chunks of banks < b come first)
                        pos0 = int(sum(U[t][b2] for b2 in range(b))) // 128
                        cols = [
                            g["tile_chunks"][t][pos0 + i][1]
                            for i in range(cap // 128)
                        ]
                        dst_cols[cols, :] = vals.reshape(-1, P)
                    toff += cap
                seg += g["blen"][b]
        # wrap idx stream per (g,b): idx k -> partition k%16, col k//16
        blocks = []
        for g in groups:
            seg = g["idx_off"]
            for b in range(NB):
                L = g["blen"][b]
                if L:
                    blocks.append(idx_all[seg : seg + L].reshape(-1, 16).T)
                seg += L
        idxw = np.tile(np.hstack(blocks), (8, 1))  # [128, TOTIDX//16]

        # host-built scatter masks for layer 0, with the dst-side dinv norm
        # baked in: masks[p, ch, d] = dinv[dst] iff the edge in slot
        # (chunk ch, partition p) targets local dst row d, else 0.
        tilebase = np.zeros(TOTCH, dtype=np.int64)
        for g in groups:
            for t in g["tiles"]:
                for (_, col) in g["tile_chunks"][t]:
                    tilebase[col] = t * P
        valid = dst_cols >= 0                              # [TOTCH, P]
        dglob = np.where(valid, c * M + tilebase[:, None] + dst_cols, 0)
        dval = (dinv[dglob] * valid).astype(np.float32)    # [TOTCH, P]
        oh = dst_cols.T[:, :, None] == np.arange(P, dtype=np.int32)[None, None, :]
        masks = (oh * dval.T[:, :, None]).astype(BF16)     # [P, TOTCH, P]

        # local dst rows for the L1 DVE-built one-hot masks
        dstl16 = np.where(valid, dst_cols, -1).astype(np.float32)

        dv = dinv[c * M : (c + 1) * M]
        dvp = np.concatenate([dv, np.ones(NT * P - M, np.float32)])
        in_maps.append(
            {
                "xT": np.ascontiguousarray(xT[:, c * M : (c + 1) * M]),
                "idx": idxw,
                "masks": masks,
                "dstloc": np.ascontiguousarray(dstl16.T).astype(BF16),
                "dinv": np.ascontiguousarray(dvp.reshape(NT, P).T),
                "iota": np.tile(np.arange(P, dtype=np.float32), (P, 1))
                .astype(BF16)
                .reshape(P, 1, P),
                "W1": np.asarray(W1, np.float32).astype(BF16),
                "W2": np.asarray(W2, np.float32).astype(BF16),
                "b1r": np.tile(np.asarray(b1, np.float32), (P, 1)),
                "b2r": np.tile(np.asarray(b2, np.float32), (P, 1)),
            }
        )
    return in_maps, U, groups, TOTIDX, TOTCH


ABLATE = frozenset()


def _build_nc(groups, TOTIDX, TOTCH, ablate=None):
    ablate = ABLATE if ablate is None else ablate
    import concourse.bacc as bacc
    import concourse.mybir as mybir
    import concourse.tile as tile
    from concourse.masks import make_identity

    F32 = mybir.dt.float32
    BF = mybir.dt.bfloat16
    I16 = mybir.dt.int16
    AOP = mybir.AluOpType

    nc = bacc.Bacc(
        "TRN2", target_bir_lowering=False, num_devices=NCORES, num_swdge_queues=4
    )
    xT_d = nc.dram_tensor("xT", [IND, M], BF, kind="ExternalInput")
    idx_d = nc.dram_tensor("idx", [P, TOTIDX // 16], I16, kind="ExternalInput")
    msk_d = nc.dram_tensor("masks", [P, TOTCH, P], BF, kind="ExternalInput")
    dst_d = nc.dram_tensor("dstloc", [P, TOTCH], BF, kind="ExternalInput")
    dinv_d = nc.dram_tensor("dinv", [P, NT], F32, kind="ExternalInput")
    iota_d = nc.dram_tensor("iota", [P, 1, P], BF, kind="ExternalInput")
    w1_d = nc.dram_tensor("W1", [IND, HID], BF, kind="ExternalInput")
    w2_d = nc.dram_tensor("W2", [HID, OUTD], BF, kind="ExternalInput")
    b1_d = nc.dram_tensor("b1r", [P, HID], F32, kind="ExternalInput")
    b2_d = nc.dram_tensor("b2r", [P, OUTD], F32, kind="ExternalInput")
    out_d = nc.dram_tensor("out", [M, OUTD], F32, kind="ExternalOutput")

    CHMAX = max(g["CHg"] for g in groups)
    NCHMAX = max(len(ck) for g in groups for ck in g["tile_chunks"].values())
    IDXWMAX = max(sum(g["blen"]) for g in groups) // 16

    with tile.TileContext(nc) as tc:
        with (
            tc.tile_pool(name="dram", bufs=1, space="DRAM") as dpool,
            tc.tile_pool(name="const", bufs=1) as cp,
            tc.tile_pool(name="resid", bufs=1) as rp,
        ):
            p_chunk = dpool.tile([M, HID], BF)
            pfull = dpool.tile([N, HID], BF, addr_space="Shared" if AG_SHARED else "Local")
            h_chunk = dpool.tile([M, HID], BF)
            hfull = dpool.tile([N, HID], BF, addr_space="Shared" if AG_SHARED else "Local")

            w1a = cp.tile([P, HID], BF)
            w1b = cp.tile([P, HID], BF)
            w2s = cp.tile([HID, OUTD], BF)
            b1r = cp.tile([P, HID], F32)
            b2r = cp.tile([P, OUTD], F32)
            iota = cp.tile([P, 1, P], BF)
            dinv = cp.tile([P, NT], F32)
            zero1 = cp.tile([P, 1], F32)
            ident = cp.tile([P, P], BF)
            nc.sync.dma_start(w1a[:], w1_d[0:P, :])
            nc.sync.dma_start(w1b[:], w1_d[P:IND, :])
            nc.sync.dma_start(w2s[:], w2_d[:])
            nc.sync.dma_start(b1r[:], b1_d[:])
            nc.sync.dma_start(b2r[:], b2_d[:])
            nc.sync.dma_start(iota[:], iota_d[:])
            nc.sync.dma_start(dinv[:], dinv_d[:])
            nc.vector.memset(zero1[:], 0.0)
            make_identity(nc, ident[:])

            # resident per-core feature copies
            # p2_sb = dinv*P' + b1  (the prescaled self term for layer 0)
            # h2_sb = dinv*h'      (the prescaled self term for layer 1)
            p2_sb = rp.tile([P, NT, HID], BF)
            h_sb = rp.tile([P, NT, HID], BF)
            h2_sb = rp.tile([P, NT, HID], BF)
            # last tile only has LAST_ROWS valid rows; zero the tail lanes
            nc.vector.memset(p2_sb[:, NT - 1, :], 0.0)
            nc.vector.memset(h_sb[:, NT - 1, :], 0.0)
            nc.vector.memset(h2_sb[:, NT - 1, :], 0.0)

            # ---------------- stage A: P' = (dinv.x) @ W1 ----------------
            for _rep in range(REPEAT):
              with (
                  nc.named_scope("stageA"),
                  tc.tile_pool(name="sa", bufs=2) as sa,
                  tc.tile_pool(name="psA", bufs=2, space="PSUM") as psA,
              ):
                  for gi_, g_ in enumerate(groups):
                      c0 = g_["tiles"][0] * P
                      c1 = min(c0 + GT * P, M)
                      cols = c1 - c0
                      xa = sa.tile([P, GT * P], BF, tag="xa")
                      xb = sa.tile([P, GT * P], BF, tag="xb")
                      nc.sync.dma_start(xa[:, :cols], xT_d[0:P, c0:c1])
                      nc.sync.dma_start(xb[:, :cols], xT_d[P:IND, c0:c1])
                      tiles = g_["tiles"]
                      slab = sa.tile([P, GT, HID], BF, tag="slab")
                      for j, t in enumerate(tiles):
                          rows = P if t < NT - 1 else LAST_ROWS
                          ps = psA.tile([P, HID], mybir.dt.float32, tag="psA")
                          nc.tensor.matmul(
                              ps[:rows, :], lhsT=xa[:, j * P : j * P + rows],
                              rhs=w1a[:], start=True, stop=False,
                          )
                          nc.tensor.matmul(
                              ps[:rows, :], lhsT=xb[:, j * P : j * P + rows],
                              rhs=w1b[:], start=False, stop=True,
                          )
                          nc.scalar.activation(
                              slab[:rows, j, :], ps[:rows, :],
                              mybir.ActivationFunctionType.Copy, bias=0.0,
                          )
                          pe = sa.tile([P, HID], mybir.dt.float32, tag="pe")
                          nc.vector.tensor_scalar(
                              out=pe[:rows, :], in0=ps[:rows, :],
                              scalar1=dinv[:rows, t : t + 1], scalar2=None,
                              op0=AOP.mult,
                          )
                          nc.vector.tensor_tensor(
                              out=p2_sb[:rows, t, :], in0=pe[:rows, :],
                              in1=b1r[:rows, :], op=AOP.add,
                          )
                      if cols == GT * P:
                          nc.sync.dma_start(
                              p_chunk[c0:c1, :].rearrange("(a p) f -> p a f", p=P),
                              slab[:],
                          )
                      else:
                          for j, t in enumerate(tiles):
                              rows = P if t < NT - 1 else LAST_ROWS
                              r0 = c0 + j * P
                              nc.sync.dma_start(
                                  p_chunk[r0 : r0 + rows, :], slab[:rows, j, :]
                              )

              if "ag" not in ablate:
                  with nc.named_scope("AG1"):
                      nc.gpsimd.collective_compute(
                          "AllGather", mybir.AluOpType.bypass,
                          replica_groups=[list(range(NCORES))],
                          ins=[p_chunk.opt()], outs=[pfull.opt()],
                      )

              # ------------- aggregation layers -------------
              with (
                  tc.tile_pool(name="agg", bufs=AGG_BUFS) as ag,
                  tc.tile_pool(name="psC", bufs=2, space="PSUM") as psC,
                  tc.tile_pool(name="psT", bufs=2, space="PSUM") as psT,
                  tc.tile_pool(name="psF", bufs=2, space="PSUM") as psF,
              ):
                  for layer in (0, 1):
                      _lsid, _ = nc.enter_named_scope(f"L{layer}", False)
                      src_full = pfull if layer == 0 else hfull
                      for gi, g in enumerate(groups):
                          W16 = sum(g["blen"]) // 16
                          io = g["idx_off"] // 16
                          idxs = ag.tile([P, IDXWMAX], I16, tag="idxs")
                          nc.scalar.dma_start(idxs[:, :W16], idx_d[:, io : io + W16])
                          mt = ag.tile([P, CHMAX, P], BF, tag="mt")
                          meng = (nc.scalar, nc.sync)[gi % 2]
                          meng.dma_start(
                              mt[:, : g["CHg"], :],
                              msk_d[:, g["dst_off"] : g["dst_off"] + g["CHg"], :],
                          )
                          gbuf = ag.tile([P, CHMAX, HID], BF, tag="gbuf")
                          boffs = np.cumsum([0] + [g["blen"][b] // 16 for b in range(NB)])
                          for b in BANK_ORDER:
                              L = g["blen"][b]
                              if L == 0 or "gather" in ablate:
                                  if L and "gather" in ablate and b == 0:
                                      nc.vector.memset(gbuf[:, :, 0:2], 0.0)
                                  continue
                              nc.gpsimd.dma_gather(
                                  gbuf[:, g["gb_choff"][b] : g["gb_choff"][b] + L // 128, :],
                                  src_full[b * BANK : (b + 1) * BANK, :],
                                  idxs[:, boffs[b] : boffs[b] + L // 16],
                                  L, L, HID, queue_num=b, single_packet=False,
                              )
                          oslab = (
                              None
                              if layer == 0
                              else ag.tile([P, GT, OUTD], mybir.dt.float32, tag="oslab")
                          )
                          for j, t in enumerate(g["tiles"]):
                              rows = P if t < NT - 1 else LAST_ROWS
                              chunks = g["tile_chunks"][t]
                              mmchunks = chunks[:1] if "mm" in ablate else chunks
                              nch_eff = len(mmchunks)
                              if layer == 0:
                                  # ps = dinv.(scatter) with the norm baked into
                                  # the host masks; h' = dinv*relu(ps + p2own)
                                  ps = psC.tile([P, HID], mybir.dt.float32, tag="psC")
                                  for k, (chk, col) in enumerate(mmchunks):
                                      nc.tensor.matmul(
                                          ps[:], lhsT=mt[:, col - g["dst_off"], :],
                                          rhs=gbuf[:, chk, :],
                                          start=(k == 0), stop=(k == nch_eff - 1),
                                      )
                                  e1 = ag.tile([P, HID], mybir.dt.float32, tag="e1")
                                  nc.vector.tensor_tensor(
                                      out=e1[:], in0=ps[:], in1=p2_sb[:, t, :],
                                      op=AOP.add,
                                  )
                                  nc.vector.tensor_scalar(
                                      out=h_sb[:, t, :], in0=e1[:],
                                      scalar1=0.0, scalar2=dinv[:, t : t + 1],
                                      op0=AOP.max, op1=AOP.mult,
                                  )
                                  nc.vector.tensor_scalar(
                                      out=h2_sb[:, t, :], in0=h_sb[:, t, :],
                                      scalar1=dinv[:, t : t + 1], scalar2=None,
                                      op0=AOP.mult,
                                  )
                              else:
                                  # masks carry dinv: ps = dinv.(scatter);
                                  # T = ps + dinv*h'own ; out = T@W2 + b2
                                  ps = psC.tile([P, P], mybir.dt.float32, tag="psC")
                                  for k, (chk, col) in enumerate(mmchunks):
                                      nc.tensor.matmul(
                                          ps[:], lhsT=mt[:, col - g["dst_off"], :],
                                          rhs=gbuf[:, chk, :],
                                          start=(k == 0), stop=(k == nch_eff - 1),
                                      )
                                  e0 = ag.tile([P, HID], BF, tag="e0")
                                  nc.vector.tensor_tensor(
                                      out=e0[:], in0=ps[:], in1=h2_sb[:, t, :],
                                      op=AOP.add,
                                  )
                                  pt = psT.tile([P, P], BF, tag="psT")
                                  nc.tensor.transpose(pt[:], e0[:], ident[:])
                                  tt = ag.tile([P, P], BF, tag="tt")
                                  nc.scalar.activation(
                                      tt[:], pt[:],
                                      mybir.ActivationFunctionType.Copy,
                                      bias=0.0,
                                  )
                                  pf = psF.tile([P, OUTD], mybir.dt.float32, tag="psF")
                                  nc.tensor.matmul(
                                      pf[:rows, :], lhsT=tt[:, :rows], rhs=w2s[:],
                                      start=True, stop=True,
                                  )
                                  nc.vector.tensor_tensor(
                                      out=oslab[:rows, j, :], in0=pf[:rows, :],
                                      in1=b2r[:rows, :], op=AOP.add,
                                  )
                          # write the group's output rows
                          c0 = g["tiles"][0] * P
                          c1 = min(c0 + GT * P, M)
                          tgt = h_chunk if layer == 0 else out_d
                          if c1 - c0 == GT * P:
                              src_slab = (
                                  h_sb[:, g["tiles"][0] : g["tiles"][0] + GT, :]
                                  if layer == 0
                                  else oslab[:]
                              )
                              nc.sync.dma_start(
                                  tgt[c0:c1, :].rearrange("(a p) f -> p a f", p=P),
                                  src_slab,
                              )
                          else:
                              for j, t in enumerate(g["tiles"]):
                                  rows = P if t < NT - 1 else LAST_ROWS
                                  r0 = c0 + j * P
                                  ssl = (
                                      h_sb[:rows, t, :]
                                      if layer == 0
                                      else oslab[:rows, j, :]
                                  )
                                  nc.sync.dma_start(tgt[r0 : r0 + rows, :], ssl)
                      nc.leave_named_scope(f"L{layer}", _lsid, False)
                      if layer == 0 and "ag" not in ablate:
                          with nc.named_scope("AG2"):
                              nc.gpsimd.collective_compute(
                                  "AllGather", mybir.AluOpType.bypass,
                                  replica_groups=[list(range(NCORES))],
                                  ins=[h_chunk.opt()], outs=[hfull.opt()],
                              )
    nc.compile()
    return nc


_CACHE = {}


def _get_compiled(x, edge_index, W1, b1, W2, b2):
    in_maps, U, groups, TOTIDX, TOTCH = _host_prep(x, edge_index, W1, b1, W2, b2)
    key = (TOTIDX, TOTCH, ABLATE, GT, AG_SHARED, AGG_BUFS, REPEAT, tuple(int(v) for v in np.asarray(U).reshape(-1)[:32]))
    if key not in _CACHE:
        _CACHE[key] = _build_nc(groups, TOTIDX, TOTCH)
    return _CACHE[key], in_maps


def kernel(x, edge_index, W1, b1, W2, b2):
    from concourse.bass_utils import run_bass_kernel_spmd

    nc, in_maps = _get_compiled(x, edge_index, W1, b1, W2, b2)
    res = run_bass_kernel_spmd(nc, in_maps, core_ids=list(range(NCORES)))
    return np.concatenate([res.results[c]["out"] for c in range(NCORES)], axis=0)

